# revision 6
# baseline (speedup 1.0000x reference)
"""Trainium2 Bass kernel for the scatter_memory delta-rule module (v2).

Computation (per batch b, head h):
  Y = X @ [W_mk|W_mv|W_mb].T            (X = mem_tokens[b], [S, D])
  k_raw, new_mv, mb_raw = per-head 64-col slices of Y
  xx  = [relu(k), relu(-k)]             ([S, 128])
  mk_j = xx * roll_j(xx), j=1..3        (mk = [S, 384], all >= 0)
  ss  = ||mk||^2, alpha = rsqrt(ss)
  num = mk @ W_mem, zmk = mk @ z        (retrieval)
  prev = num / zmk   (the reference's 1e-5 eps term is <= 1e-5 relative
                      since zmk = L1(mk) >= L2(mk) = r; dropped)
  mvg = (new_mv - prev) * sigmoid(mb_raw) * alpha
  dW  = mk.T @ mvg ;  out = W_mem + dW

v2 engine plan (vs v1): mk is transposed for retrieval with PE-array
transposes into bf16 PSUM (56 ns each) instead of DMA-xbar transposes
(1.25 us each, serialized on the Sync queue -- 71% of v1's runtime).
PSUM->SBUF copies ride scalar + gpsimd. ss comes from fused
scalar_tensor_tensor accumulate ops. GpSimd's slow window-trick chain is
gone. Outer products are software-pipelined one tile back so the
in-order tensor queue never head-of-line blocks on the vector tail.

Sharding: 8 cores = (4 batches) x (2 half-head groups of 8 heads).
Host prep: X transposed to [D, S] bf16; weight slices pre-transposed;
W_mem/z packed to a [H, 3, 128, 65] rhs. Device returns dW.T [H, 64, 384]
fp32; host transposes and adds W_mem in fp32.
"""

import numpy as np
import ml_dtypes
from contextlib import ExitStack

def _split_excess_waits(nc, max_waits=1, drain_waits=1):
    """The walrus build here encodes only ONE sync wait per instruction
    (updates are separate). Move excess waits onto prepended same-engine
    drains, one wait each."""
    from concourse import mybir

    ctr = [0]
    for f in nc.m.functions:
        for bb in f.blocks:
            il = list(bb.instructions)
            out = []
            changed = False
            for inst in il:
                si = getattr(inst, "sync_info", None)
                waits = list(si.on_wait) if si and si.on_wait else []
                ups = list(si.on_update) if si and si.on_update else []
                if len(waits) > max_waits:
                    keep = waits[:max_waits]
                    rest = waits[max_waits:]
                    for i in range(0, len(rest), drain_waits):
                        chunk = rest[i:i + drain_waits]
                        ctr[0] += 1
                        d = mybir.InstDrain(
                            name=f"waitsplit{ctr[0]}",
                            ins=[],
                            outs=[],
                            bass_is_fusable=False,
                        )
                        d.engine = inst.engine
                        d.sync_info = mybir.SyncInfo(on_wait=chunk, on_update=[])
                        out.append(d)
                    inst.sync_info = mybir.SyncInfo(on_wait=keep, on_update=ups)
                    changed = True
                out.append(inst)
            if changed:
                bb.instructions = out
    return ctr[0]

B, S, D = 4, 4096, 1024
HPC = 8            # heads per core
NCORES = 8
DK = 64            # dk per head
DKEY = 384         # 2*nu*dk
DV = 64
ST = 128           # tokens per tile
NST = S // ST      # 32
NJ = 3


def _body(ctx, tc, out_dwt, xt, wt, rhs, idin):
    import concourse.bass as bass
    from concourse import mybir

    nc = tc.nc
    bf16 = mybir.dt.bfloat16
    f32 = mybir.dt.float32
    i32 = mybir.dt.int32
    Alu = mybir.AluOpType
    Act = mybir.ActivationFunctionType

    singles = ctx.enter_context(tc.tile_pool(name="singles", bufs=1))
    xpool = ctx.enter_context(tc.tile_pool(name="xpool", bufs=3))
    work = ctx.enter_context(tc.tile_pool(name="work", bufs=2))
    tiny = ctx.enter_context(tc.tile_pool(name="tiny", bufs=2))
    kbt = ctx.enter_context(tc.tile_pool(name="kbt", bufs=1, space="PSUM"))
    vpool = ctx.enter_context(tc.tile_pool(name="vpool", bufs=1, space="PSUM"))
    rpool = ctx.enter_context(tc.tile_pool(name="rpool", bufs=1, space="PSUM"))
    dpool = ctx.enter_context(tc.tile_pool(name="dpool", bufs=1, space="PSUM"))

    # ---- resident weights (split DMAs: one writer per consumed slice) ----
    wt_sb = singles.tile([128, 8, 3 * HPC * DK], bf16)   # [p, dchunk, 1536]
    wt_r = wt.rearrange("(c p) f -> p c f", p=128)
    for d in range(8):
        for wv in range(3):
            nc.sync.dma_start(
                out=wt_sb[:, d, wv * 512:(wv + 1) * 512],
                in_=wt_r[:, d, wv * 512:(wv + 1) * 512],
            )
    rhs_sb = singles.tile([128, HPC, NJ, 65], bf16)      # [klow, h, j, 65]
    rhs_r = rhs.rearrange("h j p c -> p h j c")
    for h in range(HPC):
        for j in range(NJ):
            nc.sync.dma_start(out=rhs_sb[:, h, j, :], in_=rhs_r[:, h, j, :])
    ident = singles.tile([128, 128], bf16)
    nc.sync.dma_start(out=ident, in_=idin)

    # persistent dW.T accumulators: 4 psum banks, 2 heads each ([0:64],[64:128]).
    # Zeroed once; all outer MMs run with start=False so per-element
    # has_written bits give accumulate semantics without bank-level groups.
    dw_ps = [
        dpool.tile([128, DKEY], f32, tag=f"dw{i}", name=f"dw{i}") for i in range(4)
    ]
    for i in range(4):
        nc.vector.memset(dw_ps[i], 0.0)

    # state carried across iterations for the software-pipelined outer MM
    prev_outer = [None]

    def emit_outer():
        if prev_outer[0] is None:
            return
        mvg_p, mk_p = prev_outer[0]
        for h in range(HPC):
            nc.tensor.matmul(
                dw_ps[h // 2][64 * (h % 2):64 * (h % 2) + 64, :],
                mvg_p[:, h, :],
                mk_p[:, h, :, :],
                start=False,
                stop=False,
                skip_group_check=True,
                tile_position=(0, 64 * (h % 2)),
            )
        prev_outer[0] = None

    for st in range(NST):
        s0 = st * ST
        # ---- load X.T tile ----
        x_sb = xpool.tile([128, 8, ST], bf16)
        xt_r = xt[:, s0:s0 + ST].rearrange("(c p) s -> p c s", p=128)
        for d in range(8):
            nc.sync.dma_start(out=x_sb[:, d, :], in_=xt_r[:, d, :])

        # ---- K-wave into the shared kbt bank ----
        psK = kbt.tile([128, 512], f32, tag="kbt", name="psK")
        for d in range(8):
            nc.tensor.matmul(
                psK, x_sb[:, d, :], wt_sb[:, d, 0:512],
                start=(d == 0), stop=(d == 7),
            )

        # ---- B-wave (same bank, after relus drain psK) ----
        psB = kbt.tile([128, 512], f32, tag="kbt", name="psB")
        for d in range(8):
            nc.tensor.matmul(
                psB, x_sb[:, d, :], wt_sb[:, d, 1024:1536],
                start=(d == 0), stop=(d == 7),
            )

        # ---- outer product of the previous tile (software pipeline) ----
        emit_outer()

        # ---- V-wave (own bank) ----
        psV = vpool.tile([128, 512], f32, tag="v", name="psV")
        for d in range(8):
            nc.tensor.matmul(
                psV, x_sb[:, d, :], wt_sb[:, d, 512:1024],
                start=(d == 0), stop=(d == 7),
            )

        # ---- relus -> xx2 (duplicated [xx | xx]) ----
        xx2 = work.tile([128, HPC, 256], bf16)
        kin = psK.rearrange("p (h f) -> p h f", h=HPC)  # [128, 8, 64]
        for neg, off in ((False, 0), (True, 64)):
            dst = bass.AP(
                tensor=xx2.tensor,
                offset=xx2.offset + off,
                ap=[xx2.ap[0], [256, HPC], [128, 2], [1, 64]],
            )
            src = bass.AP(
                tensor=kin.tensor,
                offset=kin.offset,
                ap=[kin.ap[0], [64, HPC], [0, 2], [1, 64]],
            )
            if neg:
                nc.scalar.activation(dst, src, Act.Relu, scale=-1.0)
            else:
                nc.scalar.activation(dst, src, Act.Relu)

        # ---- sigmoid gate ----
        g_sb = work.tile([128, HPC, DK], bf16)
        nc.scalar.activation(
            g_sb, psB.rearrange("p (h f) -> p h f", h=HPC), Act.Sigmoid
        )

        # ---- phi products: mk_j = xx * roll_j(xx) (direct views) ----
        mk = work.tile([128, HPC, NJ, 128], bf16)
        xx_c = xx2[:, :, 128:256]
        nc.gpsimd.tensor_tensor(
            mk[:, :, 1, :], xx_c, xx2[:, :, 126:254], op=Alu.mult
        )  # j=2 (aligned)
        nc.gpsimd.tensor_tensor(
            mk[:, :, 0, :], xx_c, xx2[:, :, 127:255], op=Alu.mult
        )  # j=1 (odd offset)
        nc.vector.tensor_tensor(
            mk[:, :, 2, :], xx_c, xx2[:, :, 125:253], op=Alu.mult
        )  # j=3 (odd offset, 1x mode)

        # ---- ss = ||mk||^2: one big square then per-head accumulate ----
        sq = work.tile([128, HPC, DKEY], bf16)
        mkf = mk.rearrange("p h j k -> p h (j k)")
        nc.vector.tensor_tensor(sq, mkf, mkf, op=Alu.mult)
        junk = work.tile([128, HPC, DKEY], bf16, name="junk")
        ss = tiny.tile([128, HPC], f32)
        for h in range(HPC):
            nc.vector.tensor_scalar(
                junk[:, h, :], sq[:, h, :], 1.0, 0.0,
                op0=Alu.mult, op1=Alu.add,
                accum_out=ss[:, h:h + 1],
            )

        # ---- alpha = rsqrt(ss) (fast inverse sqrt + 1 Newton) ----
        t0 = tiny.tile([128, HPC], f32)
        nc.vector.tensor_scalar(t0, ss, 1e-20, None, op0=Alu.max)
        yv = tiny.tile([128, HPC], f32)
        sh = tiny.tile([128, HPC], f32)
        nc.vector.tensor_scalar(
            sh.bitcast(i32), t0.bitcast(i32), 1, None, op0=Alu.logical_shift_right
        )
        nc.vector.tensor_scalar(
            yv.bitcast(i32), sh.bitcast(i32), -1, 0x5F3759DF,
            op0=Alu.mult, op1=Alu.add,
        )
        aa = tiny.tile([128, HPC], f32)
        bb = tiny.tile([128, HPC], f32)
        for _ in range(1):
            nc.vector.tensor_tensor(aa, yv, yv, op=Alu.mult)
            nc.vector.tensor_tensor(bb, aa, t0, op=Alu.mult)
            nc.vector.tensor_scalar(bb, bb, -0.5, 1.5, op0=Alu.mult, op1=Alu.add)
            nc.vector.tensor_tensor(yv, yv, bb, op=Alu.mult)

        # ---- va = psV * alpha (early: frees psV for next tile) ----
        va = work.tile([128, HPC, DK], bf16)
        nc.vector.tensor_tensor(
            va, psV.rearrange("p (h f) -> p h f", h=HPC),
            yv.broadcast_to([128, HPC, DK]), op=Alu.mult
        )

        # ---- PE transposes + copies + retrieval, j-staged ----
        mkT = work.tile([128, HPC, NJ, 128], bf16)   # [klow, h, j, s]
        psR0 = rpool.tile([128, 4, 65], f32, tag="r0", name="psR0")
        psR1 = rpool.tile([128, 4, 65], f32, tag="r1", name="psR1")
        psT = []
        for j in range(NJ):
            ps = kbt.tile([128, HPC, 128], bf16, tag="kbt", name=f"psT{j}")
            psT.append(ps)
            for h in range(HPC):
                nc.tensor.transpose(ps[:, h, :], mk[:, h, j, :], ident)
            # copy to SBUF on scalar (gpsimd can't read PSUM; vector is the
            # bottleneck engine so it gets none of these)
            nc.scalar.copy(mkT[:, :, j, :], ps)
            for h in range(HPC):
                pr = psR0 if h < 4 else psR1
                nc.tensor.matmul(
                    pr[:, h % 4, :],
                    mkT[:, h, j, :],
                    rhs_sb[:, h, j, :],
                    start=(j == 0),
                    stop=(j == NJ - 1),
                )

        # ---- beta = 1/zmk; p1 = num * (beta*alpha) ----
        p1 = work.tile([128, HPC, DK], bf16)
        for i, pr in enumerate((psR0, psR1)):
            zc = tiny.tile([128, 4], f32, tag="zc", name=f"zc{i}")
            nc.vector.tensor_scalar(zc, pr[:, :, 64], 1e-9, None, op0=Alu.max)
            be = tiny.tile([128, 4], f32, tag="be", name=f"be{i}")
            nc.vector.reciprocal(be, zc)
            ba = tiny.tile([128, 4], bf16, tag="ba", name=f"ba{i}")
            nc.vector.tensor_tensor(ba, be, yv[:, 4 * i:4 * i + 4], op=Alu.mult)
            nc.vector.tensor_tensor(
                p1[:, 4 * i:4 * i + 4, :], pr[:, :, 0:64],
                ba.broadcast_to([128, 4, DK]), op=Alu.mult
            )

        # ---- mvg = (va - p1) * g ----
        m1 = work.tile([128, HPC, DK], bf16)
        nc.vector.tensor_tensor(m1, va, p1, op=Alu.subtract)
        mvg = work.tile([128, HPC, DK], bf16)
        nc.vector.tensor_tensor(mvg, m1, g_sb, op=Alu.mult)

        # outer product deferred to next iteration's tensor stream
        prev_outer[0] = (mvg, mk)

    emit_outer()

    # ---- write out dW.T (PSUM -> SBUF -> DRAM) ----
    for i in range(4):
        dwsb = work.tile([128, DKEY], f32, tag="dwsb", name=f"dwsb{i}")
        nc.vector.tensor_copy(dwsb, dw_ps[i])
        nc.sync.dma_start(
            out=out_dwt[2 * i:2 * i + 2].rearrange("h v k -> (h v) k"),
            in_=dwsb,
        )


def _build():
    import concourse.bass as bass
    import concourse.tile as tile
    from concourse import mybir

    nc = bass.Bass(trn_type="TRN2", num_devices=NCORES)
    xt = nc.dram_tensor("xt", (D, S), mybir.dt.bfloat16, kind="ExternalInput").ap()
    wt = nc.dram_tensor(
        "wt", (D, 3 * HPC * DK), mybir.dt.bfloat16, kind="ExternalInput"
    ).ap()
    rhs = nc.dram_tensor(
        "rhs", (HPC, NJ, 128, 65), mybir.dt.bfloat16, kind="ExternalInput"
    ).ap()
    idin = nc.dram_tensor(
        "ident", (128, 128), mybir.dt.bfloat16, kind="ExternalInput"
    ).ap()
    out = nc.dram_tensor(
        "dwt", (HPC, DV, DKEY), mybir.dt.float32, kind="ExternalOutput"
    ).ap()
    with tile.TileContext(nc) as tc:
        with ExitStack() as ctx:
            _body(ctx, tc, out, xt, wt, rhs, idin)
    n = _split_excess_waits(nc)
    print(f"[kernel] split {n} excess-wait chunks onto drains")
    return nc


_CACHE = {}


def _prep_core_inputs(mem_tokens, W_mk, W_mv, W_mb, W_mem, z):
    """Build the 8 per-core input maps (host-side shard + layout prep)."""
    bf = ml_dtypes.bfloat16
    ident = np.eye(128, dtype=np.float32).astype(bf)
    in_maps = []
    for c in range(NCORES):
        b = c // 2
        h0 = (c % 2) * HPC
        xt = np.ascontiguousarray(mem_tokens[b].T).astype(bf)        # [D, S]
        ws = []
        for W in (W_mk, W_mv, W_mb):
            ws.append(W[h0 * DK:(h0 + HPC) * DK, :])                 # [512, D]
        wt = np.ascontiguousarray(np.concatenate(ws, axis=0).T).astype(bf)
        rhs = np.zeros((HPC, NJ, 128, 65), dtype=np.float32)
        wm = W_mem[b, h0:h0 + HPC]                                   # [8, 384, 64]
        zz = z[b, h0:h0 + HPC]                                       # [8, 384]
        for j in range(NJ):
            rhs[:, j, :, 0:64] = wm[:, j * 128:(j + 1) * 128, :]
            rhs[:, j, :, 64] = zz[:, j * 128:(j + 1) * 128]
        in_maps.append(
            {"xt": xt, "wt": wt, "rhs": rhs.astype(bf), "ident": ident}
        )
    return in_maps


def kernel(mem_tokens, W_mk, W_mv, W_mb, W_mem, z, _want_profile=False):
    from concourse.bass_utils import run_bass_kernel_spmd

    if "nc" not in _CACHE:
        _CACHE["nc"] = _build()
    nc = _CACHE["nc"]
    in_maps = _prep_core_inputs(mem_tokens, W_mk, W_mv, W_mb, W_mem, z)
    res = run_bass_kernel_spmd(
        nc, in_maps, core_ids=list(range(NCORES)), trace=_want_profile
    )
    out = np.empty((B, 16, DKEY, DV), dtype=np.float32)
    for c in range(NCORES):
        b = c // 2
        h0 = (c % 2) * HPC
        dwt = res.results[c]["dwt"]                                  # [8, 64, 384]
        out[b, h0:h0 + HPC] = np.transpose(dwt, (0, 2, 1))
    out += W_mem.astype(np.float32)
    if _want_profile:
        return out, res
    return out


# revision 10
# speedup vs baseline: 1.5626x; 1.5626x over previous
"""Trainium2 Bass kernel for the scatter_memory delta-rule module (v2).

Computation (per batch b, head h):
  Y = X @ [W_mk|W_mv|W_mb].T            (X = mem_tokens[b], [S, D])
  k_raw, new_mv, mb_raw = per-head 64-col slices of Y
  xx  = [relu(k), relu(-k)]             ([S, 128])
  mk_j = xx * roll_j(xx), j=1..3        (mk = [S, 384], all >= 0)
  ss  = ||mk||^2, alpha = rsqrt(ss)
  num = mk @ W_mem, zmk = mk @ z        (retrieval)
  prev = num / zmk   (the reference's 1e-5 eps term is <= 1e-5 relative
                      since zmk = L1(mk) >= L2(mk) = r; dropped)
  mvg = (new_mv - prev) * sigmoid(mb_raw) * alpha
  dW  = mk.T @ mvg ;  out = W_mem + dW

v2 engine plan (vs v1): mk is transposed for retrieval with PE-array
transposes into bf16 PSUM (56 ns each) instead of DMA-xbar transposes
(1.25 us each, serialized on the Sync queue -- 71% of v1's runtime).
PSUM->SBUF copies ride scalar + gpsimd. ss comes from fused
scalar_tensor_tensor accumulate ops. GpSimd's slow window-trick chain is
gone. Outer products are software-pipelined one tile back so the
in-order tensor queue never head-of-line blocks on the vector tail.

Sharding: 8 cores = (4 batches) x (2 half-head groups of 8 heads).
Host prep: X transposed to [D, S] bf16; weight slices pre-transposed;
W_mem/z packed to a [H, 3, 128, 65] rhs. Device returns dW.T [H, 64, 384]
fp32; host transposes and adds W_mem in fp32.
"""

import numpy as np
import ml_dtypes
from contextlib import ExitStack

def _split_excess_waits(nc, max_waits=1, drain_waits=1):
    """The walrus build here encodes only ONE sync wait per instruction
    (updates are separate). Move excess waits onto prepended same-engine
    drains, one wait each."""
    from concourse import mybir

    ctr = [0]
    for f in nc.m.functions:
        for bb in f.blocks:
            il = list(bb.instructions)
            out = []
            changed = False
            for inst in il:
                si = getattr(inst, "sync_info", None)
                waits = list(si.on_wait) if si and si.on_wait else []
                ups = list(si.on_update) if si and si.on_update else []
                if len(waits) > max_waits:
                    keep = waits[:max_waits]
                    rest = waits[max_waits:]
                    for i in range(0, len(rest), drain_waits):
                        chunk = rest[i:i + drain_waits]
                        ctr[0] += 1
                        d = mybir.InstDrain(
                            name=f"waitsplit{ctr[0]}",
                            ins=[],
                            outs=[],
                            bass_is_fusable=False,
                        )
                        d.engine = inst.engine
                        d.sync_info = mybir.SyncInfo(on_wait=chunk, on_update=[])
                        out.append(d)
                    inst.sync_info = mybir.SyncInfo(on_wait=keep, on_update=ups)
                    changed = True
                out.append(inst)
            if changed:
                bb.instructions = out
    return ctr[0]

B, S, D = 4, 4096, 1024
HPC = 8            # heads per core
NCORES = 8
DK = 64            # dk per head
DKEY = 384         # 2*nu*dk
DV = 64
ST = 128           # tokens per tile
NST = S // ST      # 32
NJ = 3


def _body(ctx, tc, out_dwt, xt, wt, rhs, idin):
    import concourse.bass as bass
    from concourse import mybir

    nc = tc.nc
    bf16 = mybir.dt.bfloat16
    f32 = mybir.dt.float32
    i32 = mybir.dt.int32
    Alu = mybir.AluOpType
    Act = mybir.ActivationFunctionType

    singles = ctx.enter_context(tc.tile_pool(name="singles", bufs=1))
    xpool = ctx.enter_context(tc.tile_pool(name="xpool", bufs=3))
    work = ctx.enter_context(tc.tile_pool(name="work", bufs=2))
    tiny = ctx.enter_context(tc.tile_pool(name="tiny", bufs=2))
    kbt = ctx.enter_context(tc.tile_pool(name="kbt", bufs=1, space="PSUM"))
    ring = ctx.enter_context(tc.tile_pool(name="ring", bufs=3, space="PSUM"))
    dpool = ctx.enter_context(tc.tile_pool(name="dpool", bufs=1, space="PSUM"))

    # ---- resident weights (split DMAs: one writer per consumed slice) ----
    wt_sb = singles.tile([128, 8, 3 * HPC * DK], bf16)   # [p, dchunk, 1536]
    wt_r = wt.rearrange("(c p) f -> p c f", p=128)
    for d in range(8):
        for wv in range(3):
            nc.sync.dma_start(
                out=wt_sb[:, d, wv * 512:(wv + 1) * 512],
                in_=wt_r[:, d, wv * 512:(wv + 1) * 512],
            )
    rhs_sb = singles.tile([128, HPC, NJ, 65], bf16)      # [klow, h, j, 65]
    rhs_r = rhs.rearrange("h j p c -> p h j c")
    for h in range(HPC):
        for j in range(NJ):
            nc.sync.dma_start(out=rhs_sb[:, h, j, :], in_=rhs_r[:, h, j, :])
    ident = singles.tile([128, 128], bf16)
    nc.sync.dma_start(out=ident, in_=idin)

    # persistent dW.T accumulators: 4 psum banks, 2 heads each ([0:64],[64:128]).
    # Zeroed once; all outer MMs run with start=False so per-element
    # has_written bits give accumulate semantics without bank-level groups.
    dw_ps = [
        dpool.tile([128, DKEY], f32, tag=f"dw{i}", name=f"dw{i}") for i in range(4)
    ]
    for i in range(4):
        nc.vector.memset(dw_ps[i], 0.0)

    # state carried across iterations for the software-pipelined outer MM
    prev_outer = [None]

    def emit_outer():
        if prev_outer[0] is None:
            return
        mvg_p, mk_p = prev_outer[0]
        for h in range(HPC):
            nc.tensor.matmul(
                dw_ps[h // 2][64 * (h % 2):64 * (h % 2) + 64, :],
                mvg_p[:, h, :],
                mk_p[:, h, :, :],
                start=False,
                stop=False,
                skip_group_check=True,
                tile_position=(0, 64 * (h % 2)),
            )
        prev_outer[0] = None

    for st in range(NST):
        s0 = st * ST
        # ---- load X.T tile ----
        x_sb = xpool.tile([128, 8, ST], bf16)
        xt_r = xt[:, s0:s0 + ST].rearrange("(c p) s -> p c s", p=128)
        for d in range(8):
            nc.sync.dma_start(out=x_sb[:, d, :], in_=xt_r[:, d, :])

        # ---- K-wave into the shared kbt bank ----
        psK = kbt.tile([128, 512], f32, tag="kbt", name="psK")
        for d in range(8):
            nc.tensor.matmul(
                psK, x_sb[:, d, :], wt_sb[:, d, 0:512],
                start=(d == 0), stop=(d == 7),
            )

        # ---- B-wave (same bank, after relus drain psK) ----
        psB = kbt.tile([128, 512], f32, tag="kbt", name="psB")
        for d in range(8):
            nc.tensor.matmul(
                psB, x_sb[:, d, :], wt_sb[:, d, 1024:1536],
                start=(d == 0), stop=(d == 7),
            )

        # ---- outer product of the previous tile (software pipeline) ----
        emit_outer()

        # ---- V-wave (ring slot; drained early by va) ----
        psV = ring.tile([128, 512], f32, tag="ring", name="psV")
        for d in range(8):
            nc.tensor.matmul(
                psV, x_sb[:, d, :], wt_sb[:, d, 512:1024],
                start=(d == 0), stop=(d == 7),
            )

        # ---- relus -> xx2 (duplicated [xx | xx]) ----
        xx2 = work.tile([128, HPC, 256], bf16)
        kin = psK.rearrange("p (h f) -> p h f", h=HPC)  # [128, 8, 64]
        for neg, off in ((False, 0), (True, 64)):
            dst = bass.AP(
                tensor=xx2.tensor,
                offset=xx2.offset + off,
                ap=[xx2.ap[0], [256, HPC], [128, 2], [1, 64]],
            )
            src = bass.AP(
                tensor=kin.tensor,
                offset=kin.offset,
                ap=[kin.ap[0], [64, HPC], [0, 2], [1, 64]],
            )
            if neg:
                nc.scalar.activation(dst, src, Act.Relu, scale=-1.0)
            else:
                nc.scalar.activation(dst, src, Act.Relu)

        # ---- sigmoid gate ----
        g_sb = work.tile([128, HPC, DK], bf16)
        nc.scalar.activation(
            g_sb, psB.rearrange("p (h f) -> p h f", h=HPC), Act.Sigmoid
        )

        # ---- phi products: mk_j = xx * roll_j(xx) (direct views) ----
        mk = work.tile([128, HPC, NJ, 128], bf16)
        xx_c = xx2[:, :, 128:256]
        nc.gpsimd.tensor_tensor(
            mk[:, :, 1, :], xx_c, xx2[:, :, 126:254], op=Alu.mult
        )  # j=2 (aligned)
        nc.gpsimd.tensor_tensor(
            mk[:, :, 0, :], xx_c, xx2[:, :, 127:255], op=Alu.mult
        )  # j=1 (odd offset)
        nc.vector.tensor_tensor(
            mk[:, :, 2, :], xx_c, xx2[:, :, 125:253], op=Alu.mult
        )  # j=3 (odd offset, 1x mode)

        # ---- ss = ||mk||^2 via fused square+accumulate per head ----
        sq = work.tile([128, HPC, DKEY], bf16)
        ss = tiny.tile([128, HPC], f32)
        mkf = mk.rearrange("p h j k -> p h (j k)")
        for h in range(HPC):
            nc.vector.scalar_tensor_tensor(
                sq[:, h, :], mkf[:, h, :], 1.0, mkf[:, h, :],
                op0=Alu.mult, op1=Alu.mult,
                accum_out=ss[:, h:h + 1],
            )

        # ---- alpha = rsqrt(ss) (fast inverse sqrt + 1 Newton) ----
        t0 = tiny.tile([128, HPC], f32)
        nc.vector.tensor_scalar(t0, ss, 1e-20, None, op0=Alu.max)
        yv = tiny.tile([128, HPC], f32)
        sh = tiny.tile([128, HPC], f32)
        nc.vector.tensor_scalar(
            sh.bitcast(i32), t0.bitcast(i32), 1, None, op0=Alu.logical_shift_right
        )
        nc.vector.tensor_scalar(
            yv.bitcast(i32), sh.bitcast(i32), -1, 0x5F3759DF,
            op0=Alu.mult, op1=Alu.add,
        )
        aa = tiny.tile([128, HPC], f32)
        bb = tiny.tile([128, HPC], f32)
        for _ in range(1):
            nc.vector.tensor_tensor(aa, yv, yv, op=Alu.mult)
            nc.vector.tensor_tensor(bb, aa, t0, op=Alu.mult)
            nc.vector.tensor_scalar(bb, bb, -0.5, 1.5, op0=Alu.mult, op1=Alu.add)
            nc.vector.tensor_tensor(yv, yv, bb, op=Alu.mult)

        # ---- va = psV * alpha (early: frees psV for next tile) ----
        va = work.tile([128, HPC, DK], bf16)
        nc.vector.tensor_tensor(
            va, psV.rearrange("p (h f) -> p h f", h=HPC),
            yv.broadcast_to([128, HPC, DK]), op=Alu.mult
        )

        # ---- PE transposes + copies (ring slots overlap copy with next T) ----
        mkT = work.tile([128, HPC, NJ, 128], bf16)   # [klow, h, j, s]
        for j in range(NJ):
            ps = ring.tile([128, HPC, 128], bf16, tag="ring", name=f"psT{j}")
            for h in range(HPC):
                nc.tensor.transpose(ps[:, h, :], mk[:, h, j, :], ident)
            # copy to SBUF on scalar (gpsimd can't read PSUM; vector is the
            # bottleneck engine so it gets none of these)
            nc.scalar.copy(mkT[:, :, j, :], ps)

        # ---- retrieval: psR in ring slots ----
        psR0 = ring.tile([128, 4, 65], f32, tag="ring", name="psR0")
        psR1 = ring.tile([128, 4, 65], f32, tag="ring", name="psR1")
        for j in range(NJ):
            for h in range(HPC):
                pr = psR0 if h < 4 else psR1
                nc.tensor.matmul(
                    pr[:, h % 4, :],
                    mkT[:, h, j, :],
                    rhs_sb[:, h, j, :],
                    start=(j == 0),
                    stop=(j == NJ - 1),
                )

        # ---- beta = 1/zmk; p1 = num * (beta*alpha) ----
        p1 = work.tile([128, HPC, DK], bf16)
        for i, pr in enumerate((psR0, psR1)):
            zc = tiny.tile([128, 4], f32, tag="zc", name=f"zc{i}")
            nc.vector.tensor_scalar(zc, pr[:, :, 64], 1e-9, None, op0=Alu.max)
            be = tiny.tile([128, 4], f32, tag="be", name=f"be{i}")
            nc.vector.reciprocal(be, zc)
            ba = tiny.tile([128, 4], bf16, tag="ba", name=f"ba{i}")
            nc.vector.tensor_tensor(ba, be, yv[:, 4 * i:4 * i + 4], op=Alu.mult)
            nc.vector.tensor_tensor(
                p1[:, 4 * i:4 * i + 4, :], pr[:, :, 0:64],
                ba.broadcast_to([128, 4, DK]), op=Alu.mult
            )

        # ---- mvg = (va - p1) * g ----
        m1 = work.tile([128, HPC, DK], bf16)
        nc.vector.tensor_tensor(m1, va, p1, op=Alu.subtract)
        mvg = work.tile([128, HPC, DK], bf16)
        nc.vector.tensor_tensor(mvg, m1, g_sb, op=Alu.mult)

        # outer product deferred to next iteration's tensor stream
        prev_outer[0] = (mvg, mk)

    emit_outer()

    # ---- write out dW.T (PSUM -> SBUF -> DRAM) ----
    for i in range(4):
        dwsb = work.tile([128, DKEY], f32, tag="dwsb", name=f"dwsb{i}")
        nc.vector.tensor_copy(dwsb, dw_ps[i])
        nc.sync.dma_start(
            out=out_dwt[2 * i:2 * i + 2].rearrange("h v k -> (h v) k"),
            in_=dwsb,
        )


def _build():
    import concourse.bass as bass
    import concourse.tile as tile
    from concourse import mybir

    nc = bass.Bass(trn_type="TRN2", num_devices=NCORES)
    xt = nc.dram_tensor("xt", (D, S), mybir.dt.bfloat16, kind="ExternalInput").ap()
    wt = nc.dram_tensor(
        "wt", (D, 3 * HPC * DK), mybir.dt.bfloat16, kind="ExternalInput"
    ).ap()
    rhs = nc.dram_tensor(
        "rhs", (HPC, NJ, 128, 65), mybir.dt.bfloat16, kind="ExternalInput"
    ).ap()
    idin = nc.dram_tensor(
        "ident", (128, 128), mybir.dt.bfloat16, kind="ExternalInput"
    ).ap()
    out = nc.dram_tensor(
        "dwt", (HPC, DV, DKEY), mybir.dt.float32, kind="ExternalOutput"
    ).ap()
    with tile.TileContext(nc) as tc:
        with ExitStack() as ctx:
            _body(ctx, tc, out, xt, wt, rhs, idin)
    n = _split_excess_waits(nc)
    print(f"[kernel] split {n} excess-wait chunks onto drains")
    return nc


_CACHE = {}


def _prep_core_inputs(mem_tokens, W_mk, W_mv, W_mb, W_mem, z):
    """Build the 8 per-core input maps (host-side shard + layout prep)."""
    bf = ml_dtypes.bfloat16
    ident = np.eye(128, dtype=np.float32).astype(bf)
    in_maps = []
    for c in range(NCORES):
        b = c // 2
        h0 = (c % 2) * HPC
        xt = np.ascontiguousarray(mem_tokens[b].T).astype(bf)        # [D, S]
        ws = []
        for W in (W_mk, W_mv, W_mb):
            ws.append(W[h0 * DK:(h0 + HPC) * DK, :])                 # [512, D]
        wt = np.ascontiguousarray(np.concatenate(ws, axis=0).T).astype(bf)
        rhs = np.zeros((HPC, NJ, 128, 65), dtype=np.float32)
        wm = W_mem[b, h0:h0 + HPC]                                   # [8, 384, 64]
        zz = z[b, h0:h0 + HPC]                                       # [8, 384]
        for j in range(NJ):
            rhs[:, j, :, 0:64] = wm[:, j * 128:(j + 1) * 128, :]
            rhs[:, j, :, 64] = zz[:, j * 128:(j + 1) * 128]
        in_maps.append(
            {"xt": xt, "wt": wt, "rhs": rhs.astype(bf), "ident": ident}
        )
    return in_maps


def kernel(mem_tokens, W_mk, W_mv, W_mb, W_mem, z, _want_profile=False):
    from concourse.bass_utils import run_bass_kernel_spmd

    if "nc" not in _CACHE:
        _CACHE["nc"] = _build()
    nc = _CACHE["nc"]
    in_maps = _prep_core_inputs(mem_tokens, W_mk, W_mv, W_mb, W_mem, z)
    res = run_bass_kernel_spmd(
        nc, in_maps, core_ids=list(range(NCORES)), trace=_want_profile
    )
    out = np.empty((B, 16, DKEY, DV), dtype=np.float32)
    for c in range(NCORES):
        b = c // 2
        h0 = (c % 2) * HPC
        dwt = res.results[c]["dwt"]                                  # [8, 64, 384]
        out[b, h0:h0 + HPC] = np.transpose(dwt, (0, 2, 1))
    out += W_mem.astype(np.float32)
    if _want_profile:
        return out, res
    return out
